# revision 2
# baseline (speedup 1.0000x reference)
"""Bahdanau additive attention on 8 Trainium2 NeuronCores.

reference:
    W1_hs = enc @ W1                                  # [B,S,E]
    W2_ht = dec @ W2                                  # [B,T,E]
    scores = tanh(W1_hs[:,None] + W2_ht[:,:,None] + b)  # [B,T,S,E]
    logits = scores @ V                               # [B,T,S]
    attn = softmax(logits, axis=-1)
    ctx = attn @ enc                                  # [B,T,E]
    return (ctx, attn)

Sharding: data-parallel over batch, 2 batches per core, weights replicated.

Per-core kernel layout choices:
  * E (=512) lives on partitions as 4 chunks of 128 ("e-chunks").
  * xT[e,s] = (enc@W1)^T and ybT[e,t] = (dec@W2+b)^T are computed by PE
    from DMA'd W1/W2 and PE-transposed enc/dec.
  * The per-(t, chunk) broadcast add xT + ybT[:,t] runs on VectorE as
    tensor_scalar_add (per-partition scalar = ybT column), f16 in/out (4x mode).
  * tanh runs on ScalarE in huge [128, C*G*S] instructions (amortizes the
    ~352-cycle ACTIVATE overhead; ACT is the bottleneck engine).
  * logitsT[s,t] = sum_e V[e]*tanh[e,s] via PE matmuls: lhsT = tanh tile
    (f16 -> fast weight load), rhs = V column, PSUM-accumulated over chunks.
  * softmax over s is done WITHOUT max subtraction (|logits| <= sum|V| ~ 20,
    far inside fp32 exp range): exp on ACT -> expT[s,t] in SBUF, then
    denom[t] = expT^T @ ones, ctx_raw[t,e] = expT^T @ enc as two matmuls with
    the same stationary operand, normalized by reciprocal(denom) per-partition.
  * attn[t,s] output via one PE transpose of expT + per-partition scale.
"""

import numpy as np

import concourse.bacc as bacc
import concourse.tile as tile
from concourse import mybir
from concourse import bass_utils
from concourse.masks import make_identity

B, S, T, E, D = 16, 128, 128, 512, 512
N_CORES = 8
B_LOC = B // N_CORES      # batches per core
C = E // 128              # e-chunks
G = 16                    # t-group size for staging
NG = T // G
F32 = mybir.dt.float32
F16 = mybir.dt.float16


def build():
    nc = bacc.Bacc("TRN2", target_bir_lowering=False, debug=False,
                   num_devices=N_CORES)
    enc_d = nc.dram_tensor("enc", [B_LOC, S, E], F32, kind="ExternalInput")
    dec_d = nc.dram_tensor("dec", [B_LOC, T, D], F32, kind="ExternalInput")
    w1_d = nc.dram_tensor("W1", [E, E], F32, kind="ExternalInput")
    w2_d = nc.dram_tensor("W2", [D, E], F32, kind="ExternalInput")
    b_d = nc.dram_tensor("b", [1, 1, E], F32, kind="ExternalInput")
    v_d = nc.dram_tensor("V", [E, 1], F32, kind="ExternalInput")
    ctx_d = nc.dram_tensor("ctx", [B_LOC, T, E], F32, kind="ExternalOutput")
    attn_d = nc.dram_tensor("attn", [B_LOC, T, S], F32, kind="ExternalOutput")

    with tile.TileContext(nc) as tc:
        with (
            tc.tile_pool(name="const", bufs=1) as constp,
            tc.tile_pool(name="work", bufs=2) as workp,
            tc.tile_pool(name="stag", bufs=2) as stagp,
            tc.tile_pool(name="psA", bufs=2, space="PSUM") as psA,
            tc.tile_pool(name="psL", bufs=2, space="PSUM") as psL,
            tc.tile_pool(name="psT", bufs=2, space="PSUM") as psT,
        ):
            # ---- constants ----
            w1_sb = constp.tile([128, C, E], F32)
            nc.sync.dma_start(
                out=w1_sb, in_=w1_d.ap().rearrange("(fc p) e -> p fc e", p=128))
            w2_sb = constp.tile([128, C, E], F32)
            nc.sync.dma_start(
                out=w2_sb, in_=w2_d.ap().rearrange("(fc p) e -> p fc e", p=128))
            b_sb = constp.tile([128, C], F32)
            nc.sync.dma_start(
                out=b_sb, in_=b_d.ap().rearrange("x y (c p) -> (x y p) c", p=128))
            v_sb = constp.tile([128, C], F32)
            nc.sync.dma_start(
                out=v_sb, in_=v_d.ap().rearrange("(c p) o -> p (o c)", p=128))
            v16_sb = constp.tile([128, C], F16)
            nc.vector.tensor_copy(out=v16_sb, in_=v_sb)
            ones_sb = constp.tile([128, 1], F32)
            nc.vector.memset(ones_sb, 1.0)
            ident = constp.tile([128, 128], F32)
            make_identity(nc, ident)

            for b in range(B_LOC):
                # ---- load enc/dec ----
                enc_sb = workp.tile([128, E], F32, tag="enc")   # [s, e]
                nc.sync.dma_start(out=enc_sb, in_=enc_d.ap()[b, :, :])
                dec_sb = workp.tile([128, D], F32, tag="dec")   # [t, d]
                nc.sync.dma_start(out=dec_sb, in_=dec_d.ap()[b, :, :])

                # ---- transpose enc/dec chunks via PE ----
                encT_sb = workp.tile([128, C, S], F32, tag="encT")  # [f, fc, s]
                decT_sb = workp.tile([128, C, T], F32, tag="decT")
                for c in range(C):
                    ps_tr = psA.tile([128, 128], F32, tag="psA")
                    nc.tensor.transpose(
                        ps_tr, enc_sb[:, c * 128:(c + 1) * 128], ident)
                    nc.vector.tensor_copy(out=encT_sb[:, c, :], in_=ps_tr)
                for c in range(C):
                    ps_tr = psA.tile([128, 128], F32, tag="psA")
                    nc.tensor.transpose(
                        ps_tr, dec_sb[:, c * 128:(c + 1) * 128], ident)
                    nc.vector.tensor_copy(out=decT_sb[:, c, :], in_=ps_tr)

                # ---- xT[e,s] = W1^T-chunks @ encT ----
                ps_x = psA.tile([128, C * S], F32, tag="psA")
                for ec in range(C):
                    for fc in range(C):
                        nc.tensor.matmul(
                            ps_x[:, ec * S:(ec + 1) * S],
                            lhsT=w1_sb[:, fc, ec * 128:(ec + 1) * 128],
                            rhs=encT_sb[:, fc, :],
                            start=(fc == 0), stop=(fc == C - 1))
                xT_sb = workp.tile([128, C, S], F16, tag="xT")
                nc.vector.tensor_copy(
                    out=xT_sb.rearrange("p c s -> p (c s)"), in_=ps_x)

                # ---- ybT[e,t] = W2^T-chunks @ decT + b ----
                ps_y = psA.tile([128, C * T], F32, tag="psA")
                for ec in range(C):
                    for fc in range(C):
                        nc.tensor.matmul(
                            ps_y[:, ec * T:(ec + 1) * T],
                            lhsT=w2_sb[:, fc, ec * 128:(ec + 1) * 128],
                            rhs=decT_sb[:, fc, :],
                            start=(fc == 0), stop=(fc == C - 1))
                ybT_sb = workp.tile([128, C, T], F32, tag="ybT")
                for ec in range(C):
                    nc.vector.tensor_scalar_add(
                        out=ybT_sb[:, ec, :],
                        in0=ps_y[:, ec * T:(ec + 1) * T],
                        scalar1=b_sb[:, ec:ec + 1])

                # ---- main loop: adds + tanh + V-reduce ----
                ps_logits = psL.tile([128, T], F32, tag="logits")  # [s, t]
                for g in range(NG):
                    st_in = stagp.tile([128, C, G, S], F16, tag="stin")
                    for c in range(C):
                        for tj in range(G):
                            t = g * G + tj
                            nc.vector.tensor_scalar_add(
                                out=st_in[:, c, tj, :],
                                in0=xT_sb[:, c, :],
                                scalar1=ybT_sb[:, c, t:t + 1])
                    st_th = stagp.tile([128, C, G, S], F16, tag="stth")
                    nc.scalar.activation(
                        out=st_th, in_=st_in,
                        func=mybir.ActivationFunctionType.Tanh)
                    for tj in range(G):
                        t = g * G + tj
                        for c in range(C):
                            nc.tensor.matmul(
                                ps_logits[:, t:t + 1],
                                lhsT=st_th[:, c, tj, :],
                                rhs=v16_sb[:, c:c + 1],
                                start=(c == 0), stop=(c == C - 1))

                # ---- softmax (no max-subtraction) + outputs ----
                expT_sb = workp.tile([128, T], F32, tag="expT")  # [s, t]
                nc.scalar.activation(
                    out=expT_sb, in_=ps_logits,
                    func=mybir.ActivationFunctionType.Exp)

                ps_den = psT.tile([128, 1], F32, tag="tail")
                nc.tensor.matmul(ps_den, lhsT=expT_sb, rhs=ones_sb,
                                 start=True, stop=True)
                rden_sb = workp.tile([128, 1], F32, tag="rden")  # [t, 1]
                nc.vector.reciprocal(out=rden_sb, in_=ps_den)

                ps_ctx = psT.tile([128, E], F32, tag="tail")
                nc.tensor.matmul(ps_ctx, lhsT=expT_sb, rhs=enc_sb,
                                 start=True, stop=True)
                ctx_sb = workp.tile([128, E], F32, tag="ctxsb")
                nc.vector.tensor_scalar_mul(
                    out=ctx_sb, in0=ps_ctx, scalar1=rden_sb)
                nc.sync.dma_start(out=ctx_d.ap()[b, :, :], in_=ctx_sb)

                ps_w = psT.tile([128, S], F32, tag="tail")
                nc.tensor.transpose(ps_w, expT_sb, ident)
                attn_sb = workp.tile([128, S], F32, tag="attnsb")
                nc.vector.tensor_scalar_mul(
                    out=attn_sb, in0=ps_w, scalar1=rden_sb)
                nc.sync.dma_start(out=attn_d.ap()[b, :, :], in_=attn_sb)

    nc.finalize()
    return nc


_NC_CACHE = None


def _get_nc():
    global _NC_CACHE
    if _NC_CACHE is None:
        _NC_CACHE = build()
    return _NC_CACHE


def kernel(**inputs):
    enc = np.ascontiguousarray(np.asarray(inputs["enc"], dtype=np.float32))
    dec = np.ascontiguousarray(np.asarray(inputs["dec"], dtype=np.float32))
    w1 = np.ascontiguousarray(np.asarray(inputs["W1"], dtype=np.float32))
    w2 = np.ascontiguousarray(np.asarray(inputs["W2"], dtype=np.float32))
    bb = np.ascontiguousarray(np.asarray(inputs["b"], dtype=np.float32))
    vv = np.ascontiguousarray(np.asarray(inputs["V"], dtype=np.float32))

    nc = _get_nc()
    in_maps = []
    for i in range(N_CORES):
        lo = i * B_LOC
        in_maps.append({
            "enc": enc[lo:lo + B_LOC],
            "dec": dec[lo:lo + B_LOC],
            "W1": w1, "W2": w2, "b": bb, "V": vv,
        })
    res = bass_utils.run_bass_kernel_spmd(nc, in_maps,
                                          core_ids=list(range(N_CORES)))
    ctx = np.concatenate([r["ctx"] for r in res.results], axis=0)
    attn = np.concatenate([r["attn"] for r in res.results], axis=0)
    return ctx, attn


# revision 17
# speedup vs baseline: 7805.2190x; 7805.2190x over previous
"""Bahdanau additive attention on 8 Trainium2 NeuronCores.

reference:
    W1_hs = enc @ W1                                  # [B,S,E]
    W2_ht = dec @ W2                                  # [B,T,E]
    scores = tanh(W1_hs[:,None] + W2_ht[:,:,None] + b)  # [B,T,S,E]
    logits = scores @ V                               # [B,T,S]
    attn = softmax(logits, axis=-1)
    ctx = attn @ enc                                  # [B,T,E]
    return (ctx, attn)

Sharding: data-parallel over batch, 2 batches per core, weights replicated.

Per-core kernel layout choices:
  * E (=512) lives on partitions as 4 chunks of 128 ("e-chunks").
  * xT[e,s] = (enc@W1)^T and ybT[e,t] = (dec@W2+b)^T are computed by PE
    from DMA'd W1/W2 and PE-transposed enc/dec.
  * The per-(t, chunk) broadcast add xT + ybT[:,t] runs on VectorE as
    tensor_scalar_add (per-partition scalar = ybT column), f16 in/out (4x mode).
  * tanh runs on ScalarE in huge [128, C*G*S] instructions (amortizes the
    ~352-cycle ACTIVATE overhead).
  * logitsT[s,t] = sum_e V[e]*tanh[e,s] via PE matmuls: lhsT = tanh tile
    (f16 -> fast weight load), rhs = V column, PSUM-accumulated over chunks.
  * softmax over s is done WITHOUT max subtraction (|logits| <= sum|V| ~ 20,
    far inside fp32 exp range): exp on ACT -> expT[s,t] in SBUF, then
    denom[t] = expT^T @ ones, ctx_raw[t,e] = expT^T @ enc as two matmuls with
    the same stationary operand, normalized by reciprocal(denom) per-partition.
  * attn[t,s] output via one PE transpose of expT + per-partition scale.
"""

import numpy as np

import concourse.bass as bass
import concourse.bacc as bacc
import concourse.tile as tile
from concourse import mybir
from concourse import bass_utils
from concourse.masks import make_identity

B, S, T, E, D = 16, 128, 128, 512, 512
N_CORES = 8
B_LOC = B // N_CORES      # batches per core
C = E // 128              # e-chunks
G = 16                    # t-group size for staging
NG = T // G
ACT_SPLIT = int(__import__('os').environ.get('ACT_SPLIT', '2'))  # groups/batch whose chunk-2 adds run fused on ScalarE
F32 = mybir.dt.float32
F16 = mybir.dt.float16
AF = mybir.ActivationFunctionType


def build(reps: int = 1, loop: int = 1, parts: str = "all"):
    """Build the per-core program. reps>1 replicates the whole compute body
    statically; loop>1 wraps it in a hardware For_i loop (for benchmarking).
    parts: comma-set of {prep,adds,tanh,mm,tail} or "all" (bench bisection)."""
    nc = bacc.Bacc("TRN2", target_bir_lowering=False, debug=False,
                   num_devices=N_CORES)
    enc_d = nc.dram_tensor("enc", [B_LOC, S, E], F32, kind="ExternalInput")
    dec_d = nc.dram_tensor("dec", [B_LOC, T, D], F32, kind="ExternalInput")
    w1_d = nc.dram_tensor("W1", [E, E], F32, kind="ExternalInput")
    w2_d = nc.dram_tensor("W2", [D, E], F32, kind="ExternalInput")
    b_d = nc.dram_tensor("b", [1, 1, E], F32, kind="ExternalInput")
    v_d = nc.dram_tensor("V", [E, 1], F32, kind="ExternalInput")
    ctx_d = nc.dram_tensor("ctx", [B_LOC, T, E], F32, kind="ExternalOutput")
    attn_d = nc.dram_tensor("attn", [B_LOC, T, S], F32, kind="ExternalOutput")

    pset = (
        {"prep", "adds", "tanh", "mm", "tail"}
        if parts == "all" else set(parts.split(","))
    )

    with tile.TileContext(nc) as tc:
        with (
            tc.tile_pool(name="const", bufs=1) as constp,
            tc.tile_pool(name="work", bufs=2) as workp,
            tc.tile_pool(name="stag", bufs=int(__import__('os').environ.get("STAG_BUFS", "3"))) as stagp,
            tc.tile_pool(name="psA", bufs=1, space="PSUM") as psA,
            tc.tile_pool(name="psL", bufs=2, space="PSUM") as psL,
            tc.tile_pool(name="psT", bufs=1, space="PSUM") as psT,
            tc.tile_pool(name="psAdd", bufs=1, space="PSUM") as psAdd,
        ):
            # ---- constants ----
            w1_sb = constp.tile([128, C, E], F32)
            nc.sync.dma_start(
                out=w1_sb, in_=w1_d.ap().rearrange("(fc p) e -> p fc e", p=128))
            w2_sb = constp.tile([128, C, E], F32)
            nc.sync.dma_start(
                out=w2_sb, in_=w2_d.ap().rearrange("(fc p) e -> p fc e", p=128))
            b_sb = constp.tile([128, C], F32)
            nc.sync.dma_start(
                out=b_sb, in_=b_d.ap().rearrange("x y (c p) -> (x y p) c", p=128))
            v_sb = constp.tile([128, C], F32)
            nc.sync.dma_start(
                out=v_sb, in_=v_d.ap().rearrange("(c p) o -> p (o c)", p=128))
            v16_sb = constp.tile([128, C], F16)
            nc.vector.tensor_copy(out=v16_sb, in_=v_sb)
            ones_sb = constp.tile([128, 1], F32)
            nc.vector.memset(ones_sb, 1.0)
            ident = constp.tile([128, 128], F32)
            make_identity(nc, ident)
            ident16 = constp.tile([128, 128], F16)
            nc.vector.tensor_copy(out=ident16, in_=ident)

            def prep(b):
                """Load enc/dec, build xT (f16, +b folded later into ybT) and
                ybT (f32) for batch b. Copies run on ScalarE (idle in prep)."""
                enc_sb = workp.tile([128, E], F32, tag="enc")   # [s, e]
                nc.sync.dma_start(out=enc_sb, in_=enc_d.ap()[b, :, :])
                dec_sb = workp.tile([128, D], F32, tag="dec")   # [t, d]
                nc.sync.dma_start(out=dec_sb, in_=dec_d.ap()[b, :, :])

                encT_sb = workp.tile([128, C, S], F32, tag="encT")  # [f, fc, s]
                decT_sb = workp.tile([128, C, T], F32, tag="decT")
                for c in range(C):
                    ps_tr = psA.tile([128, 128], F32, tag="psA")
                    nc.tensor.transpose(
                        ps_tr, enc_sb[:, c * 128:(c + 1) * 128], ident)
                    nc.scalar.copy(out=encT_sb[:, c, :], in_=ps_tr)
                for c in range(C):
                    ps_tr = psA.tile([128, 128], F32, tag="psA")
                    nc.tensor.transpose(
                        ps_tr, dec_sb[:, c * 128:(c + 1) * 128], ident)
                    nc.scalar.copy(out=decT_sb[:, c, :], in_=ps_tr)

                # xT[e,s] = W1^T-chunks @ encT
                ps_x = psA.tile([128, C * S], F32, tag="psA")
                for ec in range(C):
                    for fc in range(C):
                        nc.tensor.matmul(
                            ps_x[:, ec * S:(ec + 1) * S],
                            lhsT=w1_sb[:, fc, ec * 128:(ec + 1) * 128],
                            rhs=encT_sb[:, fc, :],
                            start=(fc == 0), stop=(fc == C - 1))
                xb_sb = workp.tile([128, C, S], F16, tag="xT")
                for ec in range(C):
                    nc.scalar.add(
                        out=xb_sb[:, ec, :],
                        in_=ps_x[:, ec * S:(ec + 1) * S],
                        add=b_sb[:, ec:ec + 1])

                # ybT[e,t] = W2^T-chunks @ decT + b
                ps_y = psA.tile([128, C * T], F32, tag="psA")
                for ec in range(C):
                    for fc in range(C):
                        nc.tensor.matmul(
                            ps_y[:, ec * T:(ec + 1) * T],
                            lhsT=w2_sb[:, fc, ec * 128:(ec + 1) * 128],
                            rhs=decT_sb[:, fc, :],
                            start=(fc == 0), stop=(fc == C - 1))
                ybT_sb = workp.tile([128, C, T], F32, tag="ybT")
                nc.scalar.copy(
                    out=ybT_sb.rearrange("p c t -> p (c t)"), in_=ps_y)
                ybT16_sb = workp.tile([128, C, T], F16, tag="ybT16")
                nc.vector.tensor_copy(
                    out=ybT16_sb.rearrange("p c t -> p (c t)"),
                    in_=ybT_sb.rearrange("p c t -> p (c t)"))
                return enc_sb, xb_sb, ybT_sb, ybT16_sb

            def main_loop(b, xb_sb, ybT_sb, ybT16_sb):
                ps_logits = psL.tile([128, T], F32, tag="logits")  # [s, t]
                for g in range(NG):
                    # Chunks 0-2: DVE tensor_scalar adds + one big SBUF tanh.
                    # Chunk 3: adds on PE (identity matmul; x replicated over
                    # tj via stride-0 rhs AP, y broadcast over s likewise,
                    # summed in PSUM) + one 4-bank PSUM tanh on ScalarE.
                    act_c2 = g < ACT_SPLIT
                    ndve = 2 if act_c2 else 3
                    st_in = stagp.tile([128, 3, G, S], F16, tag="stin")
                    if "adds" in pset:
                        for c in range(ndve):
                            for tj in range(G):
                                t = g * G + tj
                                nc.vector.tensor_scalar_add(
                                    out=st_in[:, c, tj, :],
                                    in0=xb_sb[:, c, :],
                                    scalar1=ybT_sb[:, c, t:t + 1])
                    st_th = stagp.tile([128, C, G, S], F16, tag="stth")
                    if "adds" in pset:
                        ps_add = psAdd.tile([128, 4, 4, S], F32, tag="psadd")
                        xsl = xb_sb[:, 3, :]
                        x_bc = bass.AP(tensor=xsl.tensor, offset=xsl.offset,
                                       ap=[xsl.ap[0], [0, 4], [1, S]])
                        for q in range(4):
                            t0 = g * G + q * 4
                            ysl = ybT16_sb[:, 3, t0:t0 + 4]
                            y_bc = bass.AP(tensor=ysl.tensor, offset=ysl.offset,
                                           ap=[ysl.ap[0], ysl.ap[1], [0, S]])
                            nc.tensor.matmul(
                                ps_add[:, q], lhsT=ident16, rhs=x_bc,
                                start=True, stop=False)
                            nc.tensor.matmul(
                                ps_add[:, q], lhsT=ident16, rhs=y_bc,
                                start=False, stop=True)
                    if "tanh" in pset:
                        nc.scalar.activation(
                            out=st_th[:, 0:ndve], in_=st_in[:, 0:ndve],
                            func=AF.Tanh)
                        if act_c2 and "adds" in pset:
                            for tj in range(G):
                                t = g * G + tj
                                nc.scalar.activation(
                                    out=st_th[:, 2, tj, :],
                                    in_=xb_sb[:, 2, :],
                                    func=AF.Tanh,
                                    bias=ybT_sb[:, 2, t:t + 1],
                                    scale=1.0)
                        if "adds" in pset:
                            nc.scalar.activation(
                                out=st_th[:, 3].rearrange("p g s -> p (g s)"),
                                in_=ps_add.rearrange("p q t s -> p (q t s)"),
                                func=AF.Tanh)
                    if "mm" in pset:
                        for tj in range(G):
                            t = g * G + tj
                            for c in range(C):
                                nc.tensor.matmul(
                                    ps_logits[:, t:t + 1],
                                    lhsT=st_th[:, c, tj, :],
                                    rhs=v16_sb[:, c:c + 1],
                                    start=(c == 0), stop=(c == C - 1))
                return ps_logits

            def tail(b, enc_sb, ps_logits):
                expT_sb = workp.tile([128, T], F32, tag="expT")  # [s, t]
                nc.scalar.activation(out=expT_sb, in_=ps_logits, func=AF.Exp)

                ps_den = psT.tile([128, 1], F32, tag="tail")
                nc.tensor.matmul(ps_den, lhsT=expT_sb, rhs=ones_sb,
                                 start=True, stop=True)
                rden_sb = workp.tile([128, 1], F32, tag="rden")  # [t, 1]
                nc.vector.reciprocal(out=rden_sb, in_=ps_den)

                ps_ctx = psT.tile([128, E], F32, tag="tail")
                nc.tensor.matmul(ps_ctx, lhsT=expT_sb, rhs=enc_sb,
                                 start=True, stop=True)
                ctx_sb = workp.tile([128, E], F32, tag="ctxsb")
                nc.scalar.mul(out=ctx_sb, in_=ps_ctx, mul=rden_sb)
                nc.sync.dma_start(out=ctx_d.ap()[b, :, :], in_=ctx_sb)

                ps_w = psT.tile([128, S], F32, tag="tail")
                nc.tensor.transpose(ps_w, expT_sb, ident)
                attn_sb = workp.tile([128, S], F32, tag="attnsb")
                nc.scalar.mul(out=attn_sb, in_=ps_w, mul=rden_sb)
                nc.sync.dma_start(out=attn_d.ap()[b, :, :], in_=attn_sb)

            def body():
                bs = [bb for _ in range(reps) for bb in range(B_LOC)]
                preps = {}
                for i, b in enumerate(bs):
                    preps[i] = prep(b)
                for i, b in enumerate(bs):
                    enc_sb, xb_sb, ybT_sb, ybT16_sb = preps[i]
                    ps_logits = main_loop(b, xb_sb, ybT_sb, ybT16_sb)
                    if "tail" in pset and "mm" in pset:
                        tail(b, enc_sb, ps_logits)

            if loop > 1:
                with tc.For_i(0, loop, 1):
                    body()
            else:
                body()

    nc.finalize()
    return nc


_NC_CACHE = None


def _get_nc():
    global _NC_CACHE
    if _NC_CACHE is None:
        _NC_CACHE = build()
    return _NC_CACHE


def kernel(**inputs):
    enc = np.ascontiguousarray(np.asarray(inputs["enc"], dtype=np.float32))
    dec = np.ascontiguousarray(np.asarray(inputs["dec"], dtype=np.float32))
    w1 = np.ascontiguousarray(np.asarray(inputs["W1"], dtype=np.float32))
    w2 = np.ascontiguousarray(np.asarray(inputs["W2"], dtype=np.float32))
    bb = np.ascontiguousarray(np.asarray(inputs["b"], dtype=np.float32))
    vv = np.ascontiguousarray(np.asarray(inputs["V"], dtype=np.float32))

    nc = _get_nc()
    in_maps = []
    for i in range(N_CORES):
        lo = i * B_LOC
        in_maps.append({
            "enc": enc[lo:lo + B_LOC],
            "dec": dec[lo:lo + B_LOC],
            "W1": w1, "W2": w2, "b": bb, "V": vv,
        })
    res = bass_utils.run_bass_kernel_spmd(nc, in_maps,
                                          core_ids=list(range(N_CORES)))
    ctx = np.concatenate([r["ctx"] for r in res.results], axis=0)
    attn = np.concatenate([r["attn"] for r in res.results], axis=0)
    return ctx, attn
